# revision 11
# baseline (speedup 1.0000x reference)
"""Batched conv layer (im2col gather + einsum) as a Bass/Tile TRN2 kernel.

Problem: x (8,16,32,32,64) f32, kernel (8,3,3,64,128) f32
         out[b,i,oh,ow,f] = sum_{kh,kw,c} xpad[b,i,oh+kh-1,ow+kw-1,c] * kernel[b,kh,kw,c,f]
         out (8,16,32,32,128) f32
Sharding: batch dim b across 8 cores (pure data parallel, no collectives).

Per-core device layout (host prepares these):
  xp : (8 pairs, 128, 34*34) f16   partition dim packs 2 images x 64 channels;
                                   free dim is the zero-padded 34x34 image plane
  kd : (128, 9*128) f16            partition dim packs 2 copies of the 64 channels
  out: (16, 128, 1024) f16         [image, filter, position]; host casts to f32

The conv is 9 shifted matmuls accumulated in PSUM per 512-position tile:
  out[f, pos] += ktap[c, f].T @ xwin[c, pos]
Images are processed in pairs occupying PE row-groups 0-63 / 64-127 so two
K=64 matmuls run concurrently in the 128x128 array.

Perf notes (from NTFF traces):
  - DMA issue costs ~0.6-0.8us on the issuing engine + ~1.2us doorbell
    latency; pre-wake dummy DMAs are a net loss (the issue cost delays the
    real loads more than the wake saves).
  - scalar.copy (ACT) triggers a 1.3us ACT_TABLE_LOAD at the head of the
    scalar stream, so loads must NOT share the scalar engine; loads ride
    sync (x-lower, kd taps 1-4 / 5-8 split so tap sems land as pair-0
    consumes them, pairs 1-7) and vector (kd tap 0, x-upper).
  - HAM clock gate: PE runs at 1.2 GHz until ~sustained matmul activity;
    warm-up matmuls (complete start/stop groups into pair-0's banks,
    overwritten by the real start=True chains) begin the ramp while loads
    are in flight.  Gaps in PE activity appear to reset the ramp window, so
    the warm count is sized to just bridge until the first real matmul.
  - PSUM accumulation groups are PER BANK: two interleaved start/stop
    groups on different column regions of one bank corrupt each other.
    The last pair's column-split chains therefore run c=0 chains to stop
    before c=1 chains start; only the final [128,256] sliver is
    tail-critical (copy ~0.35us, store 64KB).
  - stores: f16, alternating sync/scalar HWDGE queues (gpsimd's queue is
    software-dynamic at ~70 GB/s - never use it for stores).
  - PSUM->SBUF copies run ~95 G elem/s (PSUM-read limited) on vector and
    scalar (gpsimd cannot read PSUM at all).
"""

import os

import numpy as np

import concourse.bass as bass
import concourse.mybir as mybir
from concourse import bacc
from concourse.bass_utils import run_bass_kernel_spmd
from concourse.tile import TileContext

# Static problem config (hardcoded per the harness contract)
B, I, H, W, C, F = 8, 16, 32, 32, 64, 128
KD = 3
HP = H + 2  # padded
WP = W + 2
NPOS = H * W          # 1024 output positions per image
NTILE = 512           # positions per PSUM tile (one bank)
NHALF = NPOS // NTILE  # 2
ROWS_PER_TILE = NTILE // W  # 16 output rows per tile
N_CORES = 8

MM_DTYPE = os.environ.get("CONV_MM_DTYPE", "f16")
OUT_F16 = os.environ.get("CONV_OUT_DTYPE", "f16") == "f16"
STORE_SPLIT = os.environ.get("CONV_STORE_SPLIT", "1") == "1"
COPY_SPLIT = os.environ.get("CONV_COPY_SPLIT", "1") == "1"
WARMUP_MM = int(os.environ.get("CONV_WARMUP_MM", "0"))
TAIL_COLSPLIT = os.environ.get("CONV_TAIL_COLSPLIT", "0") == "1"

_CACHED_NC = None
LAST_RESULTS = None


def _build_nc():
    nc = bacc.Bacc(trn_type="TRN2")

    mm_dt = {
        "f32": mybir.dt.float32,
        "f32r": mybir.dt.float32r,
        "bf16": mybir.dt.bfloat16,
        "f16": mybir.dt.float16,
    }[MM_DTYPE]
    in_dt = mm_dt if MM_DTYPE in ("f32r", "f16") else mybir.dt.float32
    out_dt = mybir.dt.float16 if OUT_F16 else mybir.dt.float32

    xp = nc.declare_dram_parameter("xp", [I // 2, 128, HP * WP], in_dt, isOutput=False)
    kd = nc.declare_dram_parameter("kd", [128, KD * KD * F], in_dt, isOutput=False)
    out = nc.declare_dram_parameter("out", [I, F, NPOS], out_dt, isOutput=True)

    with TileContext(nc) as tc:
        with (
            tc.tile_pool(name="kpool", bufs=1) as kpool,
            tc.tile_pool(name="xpool", bufs=8) as xpool,
            tc.tile_pool(name="opool", bufs=32) as opool,
            tc.tile_pool(name="psum", bufs=8, space="PSUM") as psum_pool,
        ):
            # Pre-allocate pair-0's psum tiles: warm-up matmuls target them
            # with complete start/stop groups; the real chains re-open the
            # banks with start=True, overwriting the garbage.
            psums0 = [[psum_pool.tile([128, NTILE], mybir.dt.float32,
                                      name=f"ps_0_{h}_{p}", tag="ps")
                       for p in range(2)] for h in range(NHALF)]

            if WARMUP_MM > 0:
                wtile = kpool.tile([128, 512], mybir.dt.float16, tag="warm_in")
                nc.gpsimd.memset(wtile[:, :], 0.0)
                for i in range(WARMUP_MM):
                    p0 = (i % 2) * 64
                    dst = psums0[0][i % 2]
                    nc.tensor.matmul(
                        dst[:, 0:256], wtile[p0:p0 + 64, 0:128],
                        wtile[p0:p0 + 64, 256:512],
                        start=True, stop=True, skip_group_check=True,
                    )

            x_dt = mybir.dt.bfloat16 if MM_DTYPE == "bf16" else in_dt

            # Loads: sync gets x-lower + kd taps 1-4 / 5-8 + pairs 1-7;
            # vector gets kd tap 0 + x-upper.  Scalar gets none (its stream
            # starts with a 1.3us ACT_TABLE_LOAD for the copy activations).
            xtiles = []
            xtile0 = xpool.tile([128, HP, WP], x_dt, tag="x")
            ktile = kpool.tile([128, KD * KD, F],
                               mybir.dt.bfloat16 if MM_DTYPE == "bf16" else in_dt)
            if MM_DTYPE == "bf16":
                nc.gpsimd.dma_start(out=ktile.rearrange("p t f -> p (t f)"), in_=kd[:, :])
                nc.gpsimd.dma_start(out=xtile0.rearrange("p h w -> p (h w)"), in_=xp[0])
            else:
                # All loads on sync, ordered by when pair-0's chains need
                # them (vector cannot issue DMAs; scalar's stream starts
                # with the 1.3us ACT_TABLE_LOAD).
                nc.sync.dma_start(
                    out=xtile0[0:64, 0:18, :].rearrange("p h w -> p (h w)"),
                    in_=xp[0, 0:64, 0:18 * WP])
                nc.sync.dma_start(out=ktile[:, 0, :], in_=kd[:, 0:F])
                nc.sync.dma_start(
                    out=xtile0[64:128, 0:18, :].rearrange("p h w -> p (h w)"),
                    in_=xp[0, 64:128, 0:18 * WP])
                nc.sync.dma_start(
                    out=ktile[:, 1:5, :].rearrange("p t f -> p (t f)"),
                    in_=kd[:, F:5 * F])
                nc.sync.dma_start(
                    out=ktile[:, 5:KD * KD, :].rearrange("p t f -> p (t f)"),
                    in_=kd[:, 5 * F:KD * KD * F])
                nc.sync.dma_start(out=xtile0[:, 18:HP, :].rearrange("p h w -> p (h w)"),
                                  in_=xp[0, :, 18 * WP:HP * WP])
            xtiles.append(xtile0)

            for pair in range(1, I // 2):
                xt = xpool.tile([128, HP, WP], x_dt, name=f"x_{pair}", tag="x")
                nc.sync.dma_start(out=xt.rearrange("p h w -> p (h w)"), in_=xp[pair])
                xtiles.append(xt)

            store_engines = [nc.sync, nc.scalar] if STORE_SPLIT else [nc.sync]
            copy_engines = [nc.vector, nc.scalar] if COPY_SPLIT else [nc.vector]

            def do_copy(eng, out_ap, in_ap):
                if eng is nc.scalar:
                    eng.copy(out=out_ap, in_=in_ap)
                else:
                    eng.tensor_copy(out=out_ap, in_=in_ap)

            def emit_mm(psums, xtile, schedule):
                # schedule: list of (half, par, t)
                for half, par, t in schedule:
                    kh, kw = divmod(t, KD)
                    oh0 = half * ROWS_PER_TILE
                    p0 = par * 64
                    lhsT = ktile[p0:p0 + 64, t, :]
                    rhs = xtile[p0:p0 + 64, oh0 + kh:oh0 + kh + ROWS_PER_TILE,
                                kw:kw + W]
                    nc.tensor.matmul(
                        psums[half][par][:, :], lhsT, rhs,
                        start=(t == 0), stop=(t == KD * KD - 1),
                    )

            def _emit_one(psums, xtile, half, par, c, t):
                kh, kw = divmod(t, KD)
                oh0 = half * ROWS_PER_TILE + c * (ROWS_PER_TILE // 2)
                p0 = par * 64
                lhsT = ktile[p0:p0 + 64, t, :]
                rhs = xtile[p0:p0 + 64, oh0 + kh:oh0 + kh + ROWS_PER_TILE // 2,
                            kw:kw + W]
                nc.tensor.matmul(
                    psums[half][par][:, c * 256:(c + 1) * 256], lhsT, rhs,
                    start=(t == 0), stop=(t == KD * KD - 1),
                    skip_group_check=True,
                )

            def emit_mm_colsplit_phase(psums, xtile, subchains):
                # subchains all on distinct banks; taps 0-6 tap-major, then
                # pairs (alternating row group) finish taps 7-8 interleaved
                # so completions stagger while staying dual-issue.
                for t in range(KD * KD - 2):
                    for half, par, c in subchains:
                        _emit_one(psums, xtile, half, par, c, t)
                for k in range(0, len(subchains), 2):
                    a, b = subchains[k], subchains[k + 1]
                    for t in (KD * KD - 2, KD * KD - 1):
                        _emit_one(psums, xtile, *a, t)
                        _emit_one(psums, xtile, *b, t)

            tile_idx = 0
            for pair in range(I // 2):
                xtile = xtiles[pair]
                last = pair == I // 2 - 1
                if pair == 0:
                    psums = psums0
                else:
                    psums = [[psum_pool.tile([128, NTILE], mybir.dt.float32,
                                             name=f"ps_{pair}_{h}_{p}", tag="ps")
                              for p in range(2)] for h in range(NHALF)]

                if pair == 0:
                    # half-major: half 0 only needs the first row-split load
                    sched = [(h, par, t) for h in range(NHALF)
                             for t in range(KD * KD) for par in range(2)]
                    emit_mm(psums, xtile, sched)
                elif last and TAIL_COLSPLIT:
                    # PSUM groups are per bank: run all c=0 chains to stop,
                    # copy them while the c=1 chains run.
                    c0 = [(0, 0, 0), (0, 1, 0), (1, 0, 0), (1, 1, 0)]
                    c1 = [(0, 0, 1), (0, 1, 1), (1, 0, 1), (1, 1, 1)]
                    emit_mm_colsplit_phase(psums, xtile, c0)
                    emit_mm_colsplit_phase(psums, xtile, c1)
                else:
                    sched = [(h, par, t) for t in range(KD * KD - 2)
                             for h in range(NHALF) for par in range(2)]
                    sched += [(h, par, t) for h in range(NHALF)
                              for par in range(2)
                              for t in (KD * KD - 2, KD * KD - 1)]
                    emit_mm(psums, xtile, sched)

                if last and TAIL_COLSPLIT:
                    otiles = {}
                    for h in range(NHALF):
                        for p in range(2):
                            otiles[(h, p)] = opool.tile(
                                [128, NTILE], out_dt,
                                name=f"o_{pair}_{h}_{p}", tag="o")
                    for k, (h, p, c) in enumerate(c0 + c1):
                        i_img = pair * 2 + p
                        ot = otiles[(h, p)]
                        ceng = copy_engines[k % len(copy_engines)]
                        seng = store_engines[k % len(store_engines)]
                        do_copy(ceng, ot[:, c * 256:(c + 1) * 256],
                                psums[h][p][:, c * 256:(c + 1) * 256])
                        seng.dma_start(
                            out=out[i_img, :,
                                    h * NTILE + c * 256:h * NTILE + (c + 1) * 256],
                            in_=ot[:, c * 256:(c + 1) * 256],
                        )
                else:
                    for half in range(NHALF):
                        for par in range(2):
                            i_img = pair * 2 + par
                            otile = opool.tile([128, NTILE], out_dt,
                                               name=f"o_{pair}_{half}_{par}", tag="o")
                            ceng = copy_engines[tile_idx % len(copy_engines)]
                            seng = store_engines[tile_idx % len(store_engines)]
                            do_copy(ceng, otile[:, :], psums[half][par][:, :])
                            seng.dma_start(
                                out=out[i_img, :, half * NTILE:(half + 1) * NTILE],
                                in_=otile[:, :],
                            )
                            tile_idx += 1
    nc.compile()
    return nc


def _prep_core_inputs(x_b: np.ndarray, k_b: np.ndarray):
    """x_b (16,32,32,64) f32, k_b (3,3,64,128) f32 -> device layouts."""
    np_in = np.float16 if MM_DTYPE == "f16" else np.float32
    xpad = np.zeros((I, HP, WP, C), dtype=np_in)
    xpad[:, 1:H + 1, 1:W + 1, :] = x_b
    # (I, HP, WP, C) -> (I, C, HP, WP) -> (I//2, 2*C, HP*WP)
    xp = np.ascontiguousarray(xpad.transpose(0, 3, 1, 2)).reshape(I // 2, 2 * C, HP * WP)

    kc = k_b.reshape(KD * KD, C, F)                       # (9, 64, 128)
    kdup = np.concatenate([kc, kc], axis=1)               # (9, 128, 128)
    kd = np.ascontiguousarray(kdup.transpose(1, 0, 2)).reshape(128, KD * KD * F)
    if MM_DTYPE == "f16":
        kd = kd.astype(np.float16)
    return {"xp": xp, "kd": kd}


def kernel(**inputs) -> np.ndarray:
    global _CACHED_NC, LAST_RESULTS
    x = np.asarray(inputs["x"], dtype=np.float32)
    k = np.asarray(inputs["kernel"], dtype=np.float32)

    if _CACHED_NC is None:
        _CACHED_NC = _build_nc()
    nc = _CACHED_NC

    in_maps = [_prep_core_inputs(x[b], k[b]) for b in range(B)]
    res = run_bass_kernel_spmd(nc, in_maps, core_ids=list(range(N_CORES)))
    LAST_RESULTS = res

    outs = []
    for b in range(B):
        o = np.asarray(res.results[b]["out"], dtype=np.float32)  # (16, 128, 1024)
        o = o.transpose(0, 2, 1).reshape(I, H, W, F)             # (16, 32, 32, 128)
        outs.append(o)
    return np.ascontiguousarray(np.stack(outs, axis=0))


# revision 21
# speedup vs baseline: 1.0879x; 1.0879x over previous
"""Batched conv layer (im2col gather + einsum) as a Bass/Tile TRN2 kernel.

Problem: x (8,16,32,32,64) f32, kernel (8,3,3,64,128) f32
         out[b,i,oh,ow,f] = sum_{kh,kw,c} xpad[b,i,oh+kh-1,ow+kw-1,c] * kernel[b,kh,kw,c,f]
         out (8,16,32,32,128) f32
Sharding: batch dim b across 8 cores (pure data parallel, no collectives).

Per-core device layout (host prepares these):
  xp : (8 pairs, 128, 34*34) f16   partition dim packs 2 images x 64 channels;
                                   free dim is the zero-padded 34x34 image plane
  kd : (128, 9*128) f16            partition dim packs 2 copies of the 64 channels
  out: (16, 128, 1024) f16         [image, filter, position]; host casts to f32

The conv is 9 shifted matmuls accumulated in PSUM per 512-position tile:
  out[f, pos] += ktap[c, f].T @ xwin[c, pos]
Images are processed in pairs occupying PE row-groups 0-63 / 64-127 so two
K=64 matmuls run concurrently in the 128x128 array.

Perf notes (from NTFF traces):
  - DMA issue costs ~0.6-0.8us on the issuing engine + ~1.2us doorbell
    latency; pre-wake dummy DMAs are a net loss (the issue cost delays the
    real loads more than the wake saves).
  - scalar.copy (ACT) triggers a 1.3us ACT_TABLE_LOAD at the head of the
    scalar stream, so loads must NOT share the scalar engine; loads ride
    sync (x-lower, kd taps 1-4 / 5-8 split so tap sems land as pair-0
    consumes them, pairs 1-7) and vector (kd tap 0, x-upper).
  - HAM clock gate: PE runs at 1.2 GHz until ~sustained matmul activity;
    warm-up matmuls (complete start/stop groups into pair-0's banks,
    overwritten by the real start=True chains) begin the ramp while loads
    are in flight.  Gaps in PE activity appear to reset the ramp window, so
    the warm count is sized to just bridge until the first real matmul.
  - PSUM accumulation groups are PER BANK: two interleaved start/stop
    groups on different column regions of one bank corrupt each other.
    The last pair's column-split chains therefore run c=0 chains to stop
    before c=1 chains start; only the final [128,256] sliver is
    tail-critical (copy ~0.35us, store 64KB).
  - stores: f16, alternating sync/scalar HWDGE queues (gpsimd's queue is
    software-dynamic at ~70 GB/s - never use it for stores).
  - PSUM->SBUF copies run ~95 G elem/s (PSUM-read limited) on vector and
    scalar (gpsimd cannot read PSUM at all).
"""

import os

import numpy as np

import concourse.bass as bass
import concourse.mybir as mybir
from concourse import bacc
from concourse.bass_utils import run_bass_kernel_spmd
from concourse.tile import TileContext

# Static problem config (hardcoded per the harness contract)
B, I, H, W, C, F = 8, 16, 32, 32, 64, 128
KD = 3
HP = H + 2  # padded
WP = W + 2
NPOS = H * W          # 1024 output positions per image
NTILE = 512           # positions per PSUM tile (one bank)
NHALF = NPOS // NTILE  # 2
ROWS_PER_TILE = NTILE // W  # 16 output rows per tile
N_CORES = 8

MM_DTYPE = os.environ.get("CONV_MM_DTYPE", "f16")
OUT_F16 = os.environ.get("CONV_OUT_DTYPE", "f16") == "f16"
STORE_SPLIT = os.environ.get("CONV_STORE_SPLIT", "1") == "1"
COPY_SPLIT = os.environ.get("CONV_COPY_SPLIT", "1") == "1"
WARMUP_MM = int(os.environ.get("CONV_WARMUP_MM", "0"))
TAIL_COLSPLIT = os.environ.get("CONV_TAIL_COLSPLIT", "0") == "1"
# pair 0 in fp8 e4m3 with DoubleRow (2 taps per matmul): the cold-clock
# phase (PE at 1.2 GHz until the HAM ramp) runs pair 0, and fp8-DR streams
# 2x the MACs per cycle.  Costs ~1.3e-2 global rel err (2 of 16 images in
# fp8), inside the 2e-2 budget.
P0_FP8 = os.environ.get("CONV_P0_FP8", "1") == "1"

_CACHED_NC = None
LAST_RESULTS = None


def _build_nc():
    nc = bacc.Bacc(trn_type="TRN2")

    mm_dt = {
        "f32": mybir.dt.float32,
        "f32r": mybir.dt.float32r,
        "bf16": mybir.dt.bfloat16,
        "f16": mybir.dt.float16,
    }[MM_DTYPE]
    in_dt = mm_dt if MM_DTYPE in ("f32r", "f16") else mybir.dt.float32
    out_dt = mybir.dt.float16 if OUT_F16 else mybir.dt.float32

    xp = nc.declare_dram_parameter("xp", [I // 2, 128, HP * WP], in_dt, isOutput=False)
    kd = nc.declare_dram_parameter("kd", [128, KD * KD * F], in_dt, isOutput=False)
    out = nc.declare_dram_parameter("out", [I, F, NPOS], out_dt, isOutput=True)
    if P0_FP8:
        xp8 = nc.declare_dram_parameter("xp8", [128, HP * WP], mybir.dt.float8e4,
                                        isOutput=False)
        kd8 = nc.declare_dram_parameter("kd8", [128, KD * KD * F], mybir.dt.float8e4,
                                        isOutput=False)

    with TileContext(nc) as tc:
        with (
            tc.tile_pool(name="kpool", bufs=1) as kpool,
            tc.tile_pool(name="xpool", bufs=8) as xpool,
            tc.tile_pool(name="opool", bufs=32) as opool,
            tc.tile_pool(name="psum", bufs=8, space="PSUM") as psum_pool,
        ):
            # Pre-allocate pair-0's psum tiles: warm-up matmuls target them
            # with complete start/stop groups; the real chains re-open the
            # banks with start=True, overwriting the garbage.
            psums0 = [[psum_pool.tile([128, NTILE], mybir.dt.float32,
                                      name=f"ps_0_{h}_{p}", tag="ps")
                       for p in range(2)] for h in range(NHALF)]

            if WARMUP_MM > 0:
                wtile = kpool.tile([128, 512], mybir.dt.float16, tag="warm_in")
                nc.gpsimd.memset(wtile[:, :], 0.0)
                for i in range(WARMUP_MM):
                    p0 = (i % 2) * 64
                    dst = psums0[0][i % 2]
                    nc.tensor.matmul(
                        dst[:, 0:256], wtile[p0:p0 + 64, 0:128],
                        wtile[p0:p0 + 64, 256:512],
                        start=True, stop=True, skip_group_check=True,
                    )

            x_dt = mybir.dt.bfloat16 if MM_DTYPE == "bf16" else in_dt

            # Loads: sync gets x-lower + kd taps 1-4 / 5-8 + pairs 1-7;
            # vector gets kd tap 0 + x-upper.  Scalar gets none (its stream
            # starts with a 1.3us ACT_TABLE_LOAD for the copy activations).
            xtiles = []
            xtile0 = None
            if not P0_FP8 or MM_DTYPE == "bf16":
                xtile0 = xpool.tile([128, HP, WP], x_dt, tag="x")
            ktile = kpool.tile([128, KD * KD, F],
                               mybir.dt.bfloat16 if MM_DTYPE == "bf16" else in_dt)
            if MM_DTYPE == "bf16":
                nc.gpsimd.dma_start(out=ktile.rearrange("p t f -> p (t f)"), in_=kd[:, :])
                nc.gpsimd.dma_start(out=xtile0.rearrange("p h w -> p (h w)"), in_=xp[0])
            elif P0_FP8:
                # Pair 0 computes in fp8; its fp8 data + fp8 weights load
                # first (tiny), then pair-1 f16, then the f16 weights split
                # so tap sems land as pair-1 consumes them, then the rest.
                xtile8 = xpool.tile([128, HP, WP], mybir.dt.float8e4, tag="x8")
                ktile8 = kpool.tile([128, KD * KD, F], mybir.dt.float8e4)
                nc.sync.dma_start(
                    out=xtile8[:, 0:18, :].rearrange("p h w -> p (h w)"),
                    in_=xp8[:, 0:18 * WP])
                nc.sync.dma_start(out=ktile8[:, 0:2, :].rearrange("p t f -> p (t f)"),
                                  in_=kd8[:, 0:2 * F])
                nc.sync.dma_start(
                    out=ktile8[:, 2:KD * KD, :].rearrange("p t f -> p (t f)"),
                    in_=kd8[:, 2 * F:KD * KD * F])
                nc.sync.dma_start(out=xtile8[:, 18:HP, :].rearrange("p h w -> p (h w)"),
                                  in_=xp8[:, 18 * WP:HP * WP])
                xtile0 = xtile8
            else:
                # All loads on sync, ordered by when pair-0's chains need
                # them (vector cannot issue DMAs; scalar's stream starts
                # with the 1.3us ACT_TABLE_LOAD).
                nc.sync.dma_start(
                    out=xtile0[0:64, 0:18, :].rearrange("p h w -> p (h w)"),
                    in_=xp[0, 0:64, 0:18 * WP])
                nc.sync.dma_start(out=ktile[:, 0, :], in_=kd[:, 0:F])
                nc.sync.dma_start(
                    out=xtile0[64:128, 0:18, :].rearrange("p h w -> p (h w)"),
                    in_=xp[0, 64:128, 0:18 * WP])
                nc.sync.dma_start(
                    out=ktile[:, 1:5, :].rearrange("p t f -> p (t f)"),
                    in_=kd[:, F:5 * F])
                nc.sync.dma_start(
                    out=ktile[:, 5:KD * KD, :].rearrange("p t f -> p (t f)"),
                    in_=kd[:, 5 * F:KD * KD * F])
                nc.sync.dma_start(out=xtile0[:, 18:HP, :].rearrange("p h w -> p (h w)"),
                                  in_=xp[0, :, 18 * WP:HP * WP])
            xtiles.append(xtile0)

            for pair in range(1, I // 2):
                xt = xpool.tile([128, HP, WP], x_dt, name=f"x_{pair}", tag="x")
                eng = nc.gpsimd if (P0_FP8 and pair == 2) else nc.sync
                eng.dma_start(out=xt.rearrange("p h w -> p (h w)"), in_=xp[pair])
                xtiles.append(xt)
                if P0_FP8 and pair == 1:
                    # f16 weights right after pair-1's x (pair-1 is the
                    # first f16 consumer); taps 0-2 first so its t0 matmuls
                    # aren't gated on the full 295KB tile.
                    nc.sync.dma_start(
                        out=ktile[:, 0:3, :].rearrange("p t f -> p (t f)"),
                        in_=kd[:, 0:3 * F])
                    nc.sync.dma_start(
                        out=ktile[:, 3:KD * KD, :].rearrange("p t f -> p (t f)"),
                        in_=kd[:, 3 * F:KD * KD * F])

            store_engines = [nc.sync, nc.scalar] if STORE_SPLIT else [nc.sync]
            copy_engines = [nc.vector, nc.scalar] if COPY_SPLIT else [nc.vector]

            def do_copy(eng, out_ap, in_ap):
                if eng is nc.scalar:
                    eng.copy(out=out_ap, in_=in_ap)
                else:
                    eng.tensor_copy(out=out_ap, in_=in_ap)

            def emit_mm(psums, xtile, schedule):
                # schedule: list of (half, par, t)
                for half, par, t in schedule:
                    kh, kw = divmod(t, KD)
                    oh0 = half * ROWS_PER_TILE
                    p0 = par * 64
                    lhsT = ktile[p0:p0 + 64, t, :]
                    rhs = xtile[p0:p0 + 64, oh0 + kh:oh0 + kh + ROWS_PER_TILE,
                                kw:kw + W]
                    nc.tensor.matmul(
                        psums[half][par][:, :], lhsT, rhs,
                        start=(t == 0), stop=(t == KD * KD - 1),
                    )

            def _emit_one(psums, xtile, half, par, c, t):
                kh, kw = divmod(t, KD)
                oh0 = half * ROWS_PER_TILE + c * (ROWS_PER_TILE // 2)
                p0 = par * 64
                lhsT = ktile[p0:p0 + 64, t, :]
                rhs = xtile[p0:p0 + 64, oh0 + kh:oh0 + kh + ROWS_PER_TILE // 2,
                            kw:kw + W]
                nc.tensor.matmul(
                    psums[half][par][:, c * 256:(c + 1) * 256], lhsT, rhs,
                    start=(t == 0), stop=(t == KD * KD - 1),
                    skip_group_check=True,
                )

            def emit_mm_p0_fp8(psums):
                # Pair 0 in fp8 e4m3 with DoubleRow: 4 tap-pair matmuls
                # (2 taps = 2 k-tiles each, 2x MACs/cycle) + 1 single-tap
                # matmul per 8-row region.  The DR rhs is a hand-built AP
                # [part, 2@delta, 8@WP, 32@1] where delta is the constant
                # in-plane offset between the paired taps' windows.
                # Regions of one bank run strictly sequentially (PSUM
                # accumulation groups are per bank).
                for half in range(NHALF):
                    for reg in range(2):
                        r0 = half * ROWS_PER_TILE + reg * (ROWS_PER_TILE // 2)
                        steps = [("dr", t) for t in (0, 2, 4, 6)]
                        steps.append(("single", KD * KD - 1))
                        for si, (kind, t) in enumerate(steps):
                            for par in range(2):
                                p0 = par * 64
                                out_ap = psums[half][par][:, reg * 256:(reg + 1) * 256]
                                kh, kw = divmod(t, KD)
                                if kind == "dr":
                                    khb, kwb = divmod(t + 1, KD)
                                    delta = (khb * WP + kwb) - (kh * WP + kw)
                                    base = xtile8[p0:p0 + 64,
                                                  r0 + kh:r0 + kh + 8, kw:kw + W]
                                    bap = base.ap
                                    rhs = bass.AP(
                                        base.tensor, base.offset,
                                        [list(bap[0]), [delta, 2],
                                         list(bap[1]), list(bap[2])])
                                    lhsT = ktile8[p0:p0 + 64, t:t + 2, :]
                                    nc.tensor.matmul(
                                        out_ap, lhsT, rhs,
                                        start=(si == 0), stop=False,
                                        perf_mode=mybir.MatmulPerfMode.DoubleRow,
                                        skip_group_check=True,
                                    )
                                else:
                                    rhs = xtile8[p0:p0 + 64,
                                                 r0 + kh:r0 + kh + 8, kw:kw + W]
                                    lhsT = ktile8[p0:p0 + 64, t, :]
                                    nc.tensor.matmul(
                                        out_ap, lhsT, rhs,
                                        start=False, stop=True,
                                        skip_group_check=True,
                                    )

            def emit_mm_colsplit_phase(psums, xtile, subchains):
                # subchains all on distinct banks; taps 0-6 tap-major, then
                # pairs (alternating row group) finish taps 7-8 interleaved
                # so completions stagger while staying dual-issue.
                for t in range(KD * KD - 2):
                    for half, par, c in subchains:
                        _emit_one(psums, xtile, half, par, c, t)
                for k in range(0, len(subchains), 2):
                    a, b = subchains[k], subchains[k + 1]
                    for t in (KD * KD - 2, KD * KD - 1):
                        _emit_one(psums, xtile, *a, t)
                        _emit_one(psums, xtile, *b, t)

            tile_idx = 0
            for pair in range(I // 2):
                xtile = xtiles[pair]
                last = pair == I // 2 - 1
                if pair == 0:
                    psums = psums0
                else:
                    psums = [[psum_pool.tile([128, NTILE], mybir.dt.float32,
                                             name=f"ps_{pair}_{h}_{p}", tag="ps")
                              for p in range(2)] for h in range(NHALF)]

                if pair == 0 and P0_FP8 and MM_DTYPE != "bf16":
                    emit_mm_p0_fp8(psums)
                elif pair == 0:
                    # half-major: half 0 only needs the first row-split load
                    sched = [(h, par, t) for h in range(NHALF)
                             for t in range(KD * KD) for par in range(2)]
                    emit_mm(psums, xtile, sched)
                elif last and TAIL_COLSPLIT:
                    # PSUM groups are per bank: run all c=0 chains to stop,
                    # copy them while the c=1 chains run.
                    c0 = [(0, 0, 0), (0, 1, 0), (1, 0, 0), (1, 1, 0)]
                    c1 = [(0, 0, 1), (0, 1, 1), (1, 0, 1), (1, 1, 1)]
                    emit_mm_colsplit_phase(psums, xtile, c0)
                    emit_mm_colsplit_phase(psums, xtile, c1)
                elif last:
                    # Taps 0-5 tap-major, then each chain finishes its last
                    # 3 taps as a trio: completions spread ~0.6us apart so
                    # the copies+stores pipeline under the trailing matmuls.
                    sched = [(h, par, t) for t in range(KD * KD - 4)
                             for h in range(NHALF) for par in range(2)]
                    for h, par in ((0, 0), (0, 1), (1, 0), (1, 1)):
                        sched += [(h, par, t) for t in range(KD * KD - 4, KD * KD)]
                    emit_mm(psums, xtile, sched)
                else:
                    sched = [(h, par, t) for t in range(KD * KD - 2)
                             for h in range(NHALF) for par in range(2)]
                    sched += [(h, par, t) for h in range(NHALF)
                              for par in range(2)
                              for t in (KD * KD - 2, KD * KD - 1)]
                    emit_mm(psums, xtile, sched)

                if last and TAIL_COLSPLIT:
                    otiles = {}
                    for h in range(NHALF):
                        for p in range(2):
                            otiles[(h, p)] = opool.tile(
                                [128, NTILE], out_dt,
                                name=f"o_{pair}_{h}_{p}", tag="o")
                    for k, (h, p, c) in enumerate(c0 + c1):
                        i_img = pair * 2 + p
                        ot = otiles[(h, p)]
                        ceng = copy_engines[k % len(copy_engines)]
                        seng = store_engines[k % len(store_engines)]
                        do_copy(ceng, ot[:, c * 256:(c + 1) * 256],
                                psums[h][p][:, c * 256:(c + 1) * 256])
                        seng.dma_start(
                            out=out[i_img, :,
                                    h * NTILE + c * 256:h * NTILE + (c + 1) * 256],
                            in_=ot[:, c * 256:(c + 1) * 256],
                        )
                else:
                    for half in range(NHALF):
                        for par in range(2):
                            i_img = pair * 2 + par
                            otile = opool.tile([128, NTILE], out_dt,
                                               name=f"o_{pair}_{half}_{par}", tag="o")
                            ceng = copy_engines[tile_idx % len(copy_engines)]
                            seng = store_engines[tile_idx % len(store_engines)]
                            do_copy(ceng, otile[:, :], psums[half][par][:, :])
                            seng.dma_start(
                                out=out[i_img, :, half * NTILE:(half + 1) * NTILE],
                                in_=otile[:, :],
                            )
                            tile_idx += 1
    nc.compile()
    return nc


def _prep_core_inputs(x_b: np.ndarray, k_b: np.ndarray):
    """x_b (16,32,32,64) f32, k_b (3,3,64,128) f32 -> device layouts."""
    np_in = np.float16 if MM_DTYPE == "f16" else np.float32
    xpad = np.zeros((I, HP, WP, C), dtype=np_in)
    xpad[:, 1:H + 1, 1:W + 1, :] = x_b
    # (I, HP, WP, C) -> (I, C, HP, WP) -> (I//2, 2*C, HP*WP)
    xp = np.ascontiguousarray(xpad.transpose(0, 3, 1, 2)).reshape(I // 2, 2 * C, HP * WP)

    kc = k_b.reshape(KD * KD, C, F)                       # (9, 64, 128)
    kdup = np.concatenate([kc, kc], axis=1)               # (9, 128, 128)
    kd = np.ascontiguousarray(kdup.transpose(1, 0, 2)).reshape(128, KD * KD * F)
    if MM_DTYPE == "f16":
        kd = kd.astype(np.float16)
    ret = {"xp": xp, "kd": kd}
    if P0_FP8:
        import ml_dtypes
        ret["xp8"] = xp[0].astype(ml_dtypes.float8_e4m3fn)
        ret["kd8"] = kd.astype(ml_dtypes.float8_e4m3fn)
    return ret


def kernel(**inputs) -> np.ndarray:
    global _CACHED_NC, LAST_RESULTS
    x = np.asarray(inputs["x"], dtype=np.float32)
    k = np.asarray(inputs["kernel"], dtype=np.float32)

    if _CACHED_NC is None:
        _CACHED_NC = _build_nc()
    nc = _CACHED_NC

    in_maps = [_prep_core_inputs(x[b], k[b]) for b in range(B)]
    res = run_bass_kernel_spmd(nc, in_maps, core_ids=list(range(N_CORES)))
    LAST_RESULTS = res

    outs = []
    for b in range(B):
        o = np.asarray(res.results[b]["out"], dtype=np.float32)  # (16, 128, 1024)
        o = o.transpose(0, 2, 1).reshape(I, H, W, F)             # (16, 32, 32, 128)
        outs.append(o)
    return np.ascontiguousarray(np.stack(outs, axis=0))
